# revision 8
# baseline (speedup 1.0000x reference)
"""Multi-head attention (B=2, S=2048, D=1024, H=16, causal-mask style) on
8 Trainium2 NeuronCores via Bass/Tile.

Sharding: heads across cores (2 heads/core).  Each core:
  - projects q/k/v for its 2 heads (full token range, both batches)
  - computes scores^T = k q^T per (batch, head) in k-major orientation
  - softmax without max-subtraction (scores are O(5), exp never overflows;
    masked entries multiplied by the 0/1 mask after exp, so they are exactly 0)
  - row sums obtained by augmenting the V stationary operand with a ones
    column inside the attn@V matmul
  - out-projection partial (Wo column slice); host sums partials + bias

Host-side pre/post: pure layout work (transposes / slicing / concat / sum).
"""

import os
import sys

import numpy as np

for _p in ("/opt/trn_rl_repo",):
    if os.path.isdir(_p) and _p not in sys.path:
        sys.path.insert(0, _p)

B = 2
S = 2048
D = 1024
H = 16
DK = 64
NCORES = 8
HPC = H // NCORES          # heads per core = 2
DD = HPC * DK              # per-core projected dim = 128
T = B * S                  # 4096 tokens
SCALE = DK ** -0.5

QB = 512                   # q-block (matmul moving free size)
KB = 128                   # k-chunk (partition dim of scores^T tiles)
NJQ = S // QB              # 4 q-blocks per batch
NIK = S // KB              # 16 k-chunks per batch
NDC = D // 128             # 8 contraction chunks for projections
NTB = T // QB              # 8 token blocks for projections
NVB = T // 128             # 32 token blocks for v layout
VAUGC = DK + 1             # v columns per (block, head): 64 v + 1 ones

_CACHE = {}


def _build_nc(split=True):
    from contextlib import ExitStack

    import concourse.bass as bass
    import concourse.tile as tile
    from concourse import mybir
    from concourse.vector_clock import ScopedClock

    f32 = mybir.dt.float32
    f32r = mybir.dt.float32r
    u8 = mybir.dt.uint8
    EXP = mybir.ActivationFunctionType.Exp
    IDENT = mybir.ActivationFunctionType.Identity

    def split_waits(nc):
        # this walrus accepts at most ONE sem-wait per instruction; hoist
        # extra waits onto NoOps emitted just before, on the same engine
        k = 0
        for f in nc.m.functions:
            for bb in f.blocks:
                out = []
                changed = False
                for ins in bb.instructions:
                    si = getattr(ins, "sync_info", None)
                    waits = list(si.on_wait) if si and si.on_wait else []
                    if len(waits) > 1:
                        changed = True
                        for w in waits[:-1]:
                            nop = mybir.InstNoOp(
                                name=f"{ins.name}-sw{k}", ins=[], outs=[]
                            )
                            k += 1
                            nop.engine = ins.engine
                            nop.debug = ins.debug
                            nop.sync_info = mybir.SyncInfo(
                                on_wait=[w], on_update=[]
                            )
                            out.append(nop)
                        ins.sync_info = mybir.SyncInfo(
                            on_wait=[waits[-1]],
                            on_update=list(si.on_update or []),
                        )
                    out.append(ins)
                if changed:
                    bb.instructions = out

    TC = tile.TileContext

    nc = bass.Bass()

    qt_d = nc.declare_dram_parameter("qt", [D, T], f32, isOutput=False)
    kt_d = nc.declare_dram_parameter("kt", [D, T], f32, isOutput=False)
    vt_d = nc.declare_dram_parameter("vt", [D, T], f32, isOutput=False)
    mask_d = nc.declare_dram_parameter("maskt", [B, S, S], u8, isOutput=False)
    wqt_d = nc.declare_dram_parameter("wqt", [D, DD], f32, isOutput=False)
    wkt_d = nc.declare_dram_parameter("wkt", [D, DD], f32, isOutput=False)
    wvt_d = nc.declare_dram_parameter("wvt", [D, DD], f32, isOutput=False)
    wot_d = nc.declare_dram_parameter("wot", [DD, D], f32, isOutput=False)
    bq_d = nc.declare_dram_parameter("bq", [DD, 1], f32, isOutput=False)
    bk_d = nc.declare_dram_parameter("bk", [DD, 1], f32, isOutput=False)
    bv_d = nc.declare_dram_parameter("bv", [DD, 1], f32, isOutput=False)
    id_d = nc.declare_dram_parameter("ident", [128, 128], f32, isOutput=False)
    ones_d = nc.declare_dram_parameter("ones1", [1, 128], f32, isOutput=False)

    attn_d = nc.declare_dram_parameter("attnt", [B, HPC, S, S], f32, isOutput=True)
    out_d = nc.declare_dram_parameter("outt", [D, T], f32, isOutput=True)

    def mm(out, lhsT, rhs, **kw):
        nc.tensor.matmul(out, lhsT.bitcast(f32r), rhs.bitcast(f32r), **kw)

    def r(ap):
        # walrus requires producers of fp32r-matmul inputs to emit fp32r
        return ap.bitcast(f32r)

    with TC(nc) as tc:
      with nc.allow_low_precision(reason="fp32r (20-bit fp32) matmul inputs"):
        with ExitStack() as top:
            consts = top.enter_context(tc.tile_pool(name="consts", bufs=1))
            persist = top.enter_context(tc.tile_pool(name="persist", bufs=1))

            wq_sb = consts.tile([128, D], f32, tag="wq")
            wk_sb = consts.tile([128, D], f32, tag="wk")
            wv_sb = consts.tile([128, D], f32, tag="wv")
            wo_sb = consts.tile([DD, D], f32, tag="wo")
            for d in range(NDC):
                nc.sync.dma_start(r(wq_sb[:, d * DD:(d + 1) * DD]), r(wqt_d[d * 128:(d + 1) * 128, :]))
                nc.sync.dma_start(r(wk_sb[:, d * DD:(d + 1) * DD]), r(wkt_d[d * 128:(d + 1) * 128, :]))
                nc.sync.dma_start(r(wv_sb[:, d * DD:(d + 1) * DD]), r(wvt_d[d * 128:(d + 1) * 128, :]))
            nc.sync.dma_start(r(wo_sb[:]), r(wot_d[:]))
            bq_sb = consts.tile([DD, 1], f32, tag="bq")
            bk_sb = consts.tile([DD, 1], f32, tag="bk")
            bv_sb = consts.tile([DD, 1], f32, tag="bv")
            nc.sync.dma_start(bq_sb[:], bq_d[:])
            nc.sync.dma_start(bk_sb[:], bk_d[:])
            nc.sync.dma_start(bv_sb[:], bv_d[:])
            id_sb = consts.tile([128, 128], f32, tag="id")
            ones_sb = consts.tile([1, 128], f32, tag="ones")
            nc.sync.dma_start(id_sb[:], id_d[:])
            nc.sync.dma_start(r(ones_sb[:]), r(ones_d[:]))

            # persistent activations
            q_sb = persist.tile([DD, T], f32, tag="qs")      # q^T  [dd, t]
            k_sb = persist.tile([DD, T], f32, tag="ks")      # k^T  [dd, t]
            vaug = persist.tile([128, NVB * HPC * VAUGC], f32, tag="va")
            ctx_sb = persist.tile([DD, T], f32, tag="cs")    # ctx^T [dd, t]

            # ones column of the augmented v stationary operand
            nc.vector.memset(
                vaug[:].rearrange("p (n c) -> p n c", c=VAUGC)[:, :, DK:], 1.0
            )

            # ---- stage A: projections -------------------------------------
            with ExitStack() as sa:
                ain = sa.enter_context(tc.tile_pool(name="ain", bufs=3))
                aps = sa.enter_context(tc.tile_pool(name="aps", bufs=2, space="PSUM"))
                tps = sa.enter_context(tc.tile_pool(name="tps", bufs=2, space="PSUM"))
                vtmp = sa.enter_context(tc.tile_pool(name="vtmp", bufs=2))

                for tb in range(NTB):
                    ts = slice(tb * QB, (tb + 1) * QB)
                    q_ps = aps.tile([DD, QB], f32, tag="qp")
                    k_ps = aps.tile([DD, QB], f32, tag="kp")
                    v_ps = aps.tile([DD, QB], f32, tag="vp")
                    for d in range(NDC):
                        ds_ = slice(d * 128, (d + 1) * 128)
                        qin = ain.tile([128, QB], f32, tag="qi")
                        kin = ain.tile([128, QB], f32, tag="ki")
                        vin = ain.tile([128, QB], f32, tag="vi")
                        nc.sync.dma_start(r(qin[:]), r(qt_d[ds_, ts]))
                        nc.sync.dma_start(r(kin[:]), r(kt_d[ds_, ts]))
                        nc.sync.dma_start(r(vin[:]), r(vt_d[ds_, ts]))
                        st = dict(start=(d == 0), stop=(d == NDC - 1))
                        mm(q_ps[:], wq_sb[:, d * DD:(d + 1) * DD], qin[:], **st)
                        mm(k_ps[:], wk_sb[:, d * DD:(d + 1) * DD], kin[:], **st)
                        mm(v_ps[:], wv_sb[:, d * DD:(d + 1) * DD], vin[:], **st)
                    nc.scalar.activation(r(q_sb[:, ts]), q_ps[:], IDENT, bias=bq_sb[:])
                    nc.scalar.activation(r(k_sb[:, ts]), k_ps[:], IDENT, bias=bk_sb[:])
                    vt_sb = vtmp.tile([DD, QB], f32, tag="vt")
                    nc.scalar.activation(vt_sb[:], v_ps[:], IDENT, bias=bv_sb[:])
                    # transpose v^T -> v (token-major) into the augmented layout
                    for j in range(QB // 128):
                        jb = tb * (QB // 128) + j
                        t_ps = tps.tile([128, 128], f32, tag="tp")
                        nc.tensor.transpose(
                            t_ps[:], vt_sb[:, j * 128:(j + 1) * 128], id_sb[:]
                        )
                        for lh in range(HPC):
                            base = (jb * HPC + lh) * VAUGC
                            nc.vector.tensor_copy(
                                r(vaug[:, base:base + DK]),
                                t_ps[:, lh * DK:(lh + 1) * DK],
                            )

            # ---- stage B + C: attention and out-projection ----------------
            with ExitStack() as sb:
                msk = sb.enter_context(tc.tile_pool(name="msk", bufs=6))
                pm = sb.enter_context(tc.tile_pool(name="pm", bufs=NIK + 4))
                bcp = sb.enter_context(tc.tile_pool(name="bcp", bufs=2))
                sml = sb.enter_context(tc.tile_pool(name="sml", bufs=2))
                osb = sb.enter_context(tc.tile_pool(name="osb", bufs=3))
                sps = sb.enter_context(tc.tile_pool(name="sps", bufs=3, space="PSUM"))
                cps = sb.enter_context(tc.tile_pool(name="cps", bufs=2, space="PSUM"))
                bps = sb.enter_context(tc.tile_pool(name="bps", bufs=1, space="PSUM"))
                ops = sb.enter_context(tc.tile_pool(name="ops", bufs=2, space="PSUM"))

                for b in range(B):
                    for lh in range(HPC):
                        hsl = slice(lh * DK, (lh + 1) * DK)
                        for jq in range(NJQ):
                            qs = slice(b * S + jq * QB, b * S + (jq + 1) * QB)
                            qsl = slice(jq * QB, (jq + 1) * QB)  # batch-local
                            ctx_ps = cps.tile([DK + 1, QB], f32, tag="cp")
                            pms = []
                            for ik in range(NIK):
                                ksl = slice(ik * KB, (ik + 1) * KB)
                                kts = slice(b * S + ik * KB, b * S + (ik + 1) * KB)
                                m_t = msk.tile([KB, QB], u8, tag="m")
                                nc.sync.dma_start(m_t[:], mask_d[b, ksl, qsl])
                                s_ps = sps.tile([KB, QB], f32, tag="sp")
                                mm(s_ps[:], k_sb[hsl, kts], q_sb[hsl, qs])
                                p_t = pm.tile([KB, QB], f32, tag="p")
                                nc.scalar.activation(r(p_t[:]), s_ps[:], EXP)
                                nc.vector.tensor_mul(r(p_t[:]), p_t[:], m_t[:])
                                jb = b * NIK + ik
                                base = (jb * HPC + lh) * VAUGC
                                mm(
                                    ctx_ps[:],
                                    vaug[:, base:base + VAUGC],
                                    p_t[:],
                                    start=(ik == 0),
                                    stop=(ik == NIK - 1),
                                )
                                pms.append(p_t)
                            recip = sml.tile([1, QB], f32, tag="r")
                            nc.vector.reciprocal(r(recip[:]), ctx_ps[DK:DK + 1, :])
                            bc_ps = bps.tile([128, QB], f32, tag="bp")
                            mm(bc_ps[:], ones_sb[:], recip[:])
                            bc_sb = bcp.tile([128, QB], f32, tag="bc")
                            nc.scalar.copy(bc_sb[:], bc_ps[:])
                            nc.vector.tensor_mul(
                                r(ctx_sb[hsl, qs]), ctx_ps[0:DK, :], bc_sb[0:DK, :]
                            )
                            for ik in range(NIK):
                                ksl = slice(ik * KB, (ik + 1) * KB)
                                nc.vector.tensor_mul(r(pms[ik][:]), pms[ik][:], bc_sb[:])
                                nc.sync.dma_start(
                                    attn_d[b, lh, ksl, qsl], pms[ik][:]
                                )
                    # stage C for this batch
                    for jq in range(NJQ):
                        qs = slice(b * S + jq * QB, b * S + (jq + 1) * QB)
                        for dm in range(D // 128):
                            o_ps = ops.tile([128, QB], f32, tag="op")
                            mm(o_ps[:], wo_sb[:, dm * 128:(dm + 1) * 128], ctx_sb[:, qs])
                            o_sb = osb.tile([128, QB], f32, tag="ob")
                            nc.scalar.copy(o_sb[:], o_ps[:])
                            nc.sync.dma_start(out_d[dm * 128:(dm + 1) * 128, qs], o_sb[:])

    if split:
        split_waits(nc)
    return nc


def _get_nc(split=True):
    key = ("nc", split)
    if key not in _CACHE:
        _CACHE[key] = _build_nc(split)
    return _CACHE[key]


def kernel(Q, K, V, mask, Wq, bq, Wk, bk, Wv, bv, Wo, bo):
    from concourse.bass_utils import run_bass_kernel_spmd

    nc = _get_nc()

    Qf = np.asarray(Q, np.float32).reshape(T, D)
    Kf = np.asarray(K, np.float32).reshape(T, D)
    Vf = np.asarray(V, np.float32).reshape(T, D)
    qt = np.ascontiguousarray(Qf.T)
    kt = np.ascontiguousarray(Kf.T)
    vt = np.ascontiguousarray(Vf.T)
    maskt = np.ascontiguousarray(
        np.asarray(mask).astype(np.uint8).transpose(0, 2, 1)
    )
    ident = np.eye(128, dtype=np.float32)
    ones1 = np.ones((1, 128), np.float32)

    Wq = np.asarray(Wq, np.float32)
    Wk = np.asarray(Wk, np.float32)
    Wv = np.asarray(Wv, np.float32)
    Wo = np.asarray(Wo, np.float32)
    bq = np.asarray(bq, np.float32)
    bk = np.asarray(bk, np.float32)
    bv = np.asarray(bv, np.float32)
    bo = np.asarray(bo, np.float32)

    in_maps = []
    for c in range(NCORES):
        rs = slice(c * DD, (c + 1) * DD)
        in_maps.append({
            "qt": qt,
            "kt": kt,
            "vt": vt,
            "maskt": maskt,
            "wqt": np.ascontiguousarray((Wq[rs] * SCALE).T),
            "wkt": np.ascontiguousarray(Wk[rs].T),
            "wvt": np.ascontiguousarray(Wv[rs].T),
            "wot": np.ascontiguousarray(Wo[:, rs].T),
            "bq": (bq[rs] * SCALE).reshape(DD, 1).copy(),
            "bk": bk[rs].reshape(DD, 1).copy(),
            "bv": bv[rs].reshape(DD, 1).copy(),
            "ident": ident,
            "ones1": ones1,
        })

    res = run_bass_kernel_spmd(nc, in_maps, list(range(NCORES)), trace=False)

    attn = np.empty((B, H, S, S), np.float32)
    outt_sum = np.zeros((D, T), np.float32)
    for c in range(NCORES):
        r = res.results[c]
        for lh in range(HPC):
            for b in range(B):
                attn[b, c * HPC + lh] = r["attnt"][b, lh].T
        outt_sum += r["outt"]
    out = np.ascontiguousarray(outt_sum.T).reshape(B, S, D) + bo

    return out, attn
